# revision 35
# baseline (speedup 1.0000x reference)
"""Trainium2 Bass kernel for nn_CombinatorialPathGate (single-token MoE routing).

Strategy (8 NeuronCores, tensor-parallel over the output dim):
  - Each core owns a 512-row slice of the output.  It reads its slice of
    gate_w (8 MB) and, after computing the router argmax on-device, its
    slice of the winning expert's weights (8 MB) via a dynamic-offset DMA.
  - Host pre-slices all tensors per-core so the compiled program is
    identical (SPMD) on all 8 cores; the only runtime dynamism is the
    expert index.
  - The GEMV keeps weights in their natural [row, col] layout: each
    [128, 4096] block is combined with a partition-broadcast copy of x
    using a single fused DVE tensor_tensor_reduce (multiply + free-dim
    add reduction), giving 128 dot products per instruction.
  - x arrives host-broadcast to [128, H] (one 2MB DMA); weight matrices
    stream as coalesced 4MB DMAs with a tapered 1MB tail on the expert
    side to shorten the post-DMA drain.
"""

import numpy as np

import concourse.bass as bass
import concourse.mybir as mybir
import concourse.tile as tile
from concourse.bass_utils import run_bass_kernel_spmd
from concourse.masks import make_identity

H = 4096
E = 8
NCORES = 8
S = H // NCORES      # 512 output rows per core
NB = S // 128        # 4 blocks of 128 rows
SEED = 128           # partitions of x provided by host (full broadcast)
F32 = mybir.dt.float32

_CACHE = {}

# test.py can read these after a call for profiling info
LAST_RESULTS = None


def _legalize_single_wait(nc):
    """The pinned walrus build only encodes ONE sync-wait per instruction
    ("Too many sync wait commands" otherwise).  Tile's scheduler freely
    attaches several.  Hoist all but the last wait of each instruction onto
    single-wait NoOp carriers placed immediately before it on the same
    engine — identical semantics (sequencer blocks on each in turn)."""
    n_nops = 0
    for fn in nc.m.functions:
        for blk in fn.blocks:
            new = []
            for inst in blk.instructions:
                try:
                    si = inst.sync_info
                except AttributeError:
                    si = None
                if si is not None and len(si.on_wait) > 1:
                    waits = list(si.on_wait)
                    for w in waits[:-1]:
                        nop = mybir.InstEventSemaphore(name=f"legalw-{n_nops}")
                        n_nops += 1
                        nop.engine = inst.engine
                        nop.sync_info = mybir.SyncInfo(on_wait=[w], on_update=[])
                        new.append(nop)
                    inst.sync_info = mybir.SyncInfo(
                        on_wait=[waits[-1]], on_update=list(si.on_update)
                    )
                if si is not None and len(si.on_update) > 1:
                    raise AssertionError(
                        f"multi-update instruction {inst.name}: updates cannot "
                        "be hoisted safely (async completion)"
                    )
                new.append(inst)
            blk.instructions = new
    return nc


def _build_program(legalize=True):
    nc = bass.Bass("TRN2", num_devices=NCORES)

    x32_d = nc.dram_tensor("x32_in", [SEED, H + 8], F32, kind="ExternalInput")
    rw_d = nc.dram_tensor("rw_in", [E, H + 8], F32, kind="ExternalInput")
    gw_d = nc.dram_tensor("gw_in", [S, H], F32, kind="ExternalInput")
    ew_d = nc.dram_tensor("ew_in", [E * S, H], F32, kind="ExternalInput")
    ebs_d = nc.dram_tensor("ebs_in", [E * 128, NB], F32, kind="ExternalInput")
    xs_d = nc.dram_tensor("xs_in", [128, NB], F32, kind="ExternalInput")
    gbs_d = nc.dram_tensor("gbs_in", [128, NB], F32, kind="ExternalInput")
    yc_d = nc.dram_tensor("yc_out", [128, NB], F32, kind="ExternalOutput")

    mult = mybir.AluOpType.mult
    add = mybir.AluOpType.add

    with tile.TileContext(nc) as tc:
        with (
            tc.tile_pool(name="we", bufs=4) as wepool,
            tc.tile_pool(name="pp", bufs=3) as ppool,
            tc.tile_pool(name="c", bufs=1) as cpool,
            tc.tile_pool(name="ps", bufs=1, space="PSUM") as pspool,
        ):
            # [p, r, k] view of weight matrices: t[p, r, k] = W[r*128 + p, k]
            gw_v = gw_d.ap().rearrange("(r p) k -> p r k", p=128)
            ew_v = ew_d.ap().rearrange("(r p) k -> p r k", p=128)

            # ---- x broadcast to all partitions (host-prepared), one DMA ----
            # (top priority: router + every weight-block multiply needs it.
            #  Column H of x32/rw carries 1.0 / router_b so the router bias
            #  is folded into the GEMV; cols H+1..H+7 are zero padding.)
            x_bc = cpool.tile([128, H + 8], F32)
            rw_sb = cpool.tile([E, H + 8], F32)
            # argmax weights [E-1 .. 0] via iota (no DMA dependency)
            cv_i = cpool.tile([1, E], mybir.dt.int32)
            nc.gpsimd.iota(cv_i[:], pattern=[[-1, E]], base=E - 1,
                           channel_multiplier=0)
            cv_sb = cpool.tile([1, E], F32)
            nc.vector.tensor_copy(cv_sb[:], cv_i[:])
            with tc.high_priority():
                nc.scalar.dma_start(out=rw_sb[:], in_=rw_d.ap())
                nc.sync.dma_start(out=x_bc[:], in_=x32_d.ap())
            xs_sb = cpool.tile([128, NB], F32)
            nc.scalar.dma_start(out=xs_sb[:], in_=xs_d.ap())
            gbs_sb = cpool.tile([128, NB], F32)
            nc.scalar.dma_start(out=gbs_sb[:], in_=gbs_d.ap())

            # ---- router: logits[e] = sum_k rw[e,k] * x[k] ----
            # (DVE multiply, then ACT copy-with-accumulate reduces free dim.
            #  The whole chain down to the Pool register load is
            #  high-priority so the scheduler doesn't starve it behind the
            #  4.3 us gate-block multiplies — the expert DMAs wait on it.)
            with tc.high_priority():
                rprod = ppool.tile([128, H + 8], F32, tag="prod")
                nc.vector.tensor_mul(rprod[0:E, :], rw_sb[:], x_bc[0:E, :])
                logits8 = cpool.tile([E, 1], F32)
                nc.scalar.activation(
                    rprod[0:E, :], rprod[0:E, :],
                    mybir.ActivationFunctionType.Copy,
                    accum_out=logits8[:],
                )
                # transpose [8,1] -> [1,8] on the otherwise-idle tensor
                # engine (PSUM result read directly by the DVE chain) — a DMA
                # here would queue behind the multi-MB weight transfers.
                ident = cpool.tile([E, E], F32)
                make_identity(nc, ident[:])
                lrow_pre = pspool.tile([1, E], F32)
                nc.tensor.transpose(out=lrow_pre[:], in_=logits8[:], identity=ident[:])
                mx = mybir.AluOpType.max

                def max_tree(dst_pool, src):
                    # free-dim max of [1, 8] via 3 pairwise-max steps
                    t4 = dst_pool.tile([1, 4], F32, tag="amx4")
                    nc.vector.tensor_tensor(
                        out=t4[:], in0=src[0:1, 0:4], in1=src[0:1, 4:8], op=mx
                    )
                    t2 = dst_pool.tile([1, 2], F32, tag="amx2")
                    nc.vector.tensor_tensor(
                        out=t2[:], in0=t4[0:1, 0:2], in1=t4[0:1, 2:4], op=mx
                    )
                    t1 = dst_pool.tile([1, 1], F32, tag="amx1")
                    nc.vector.tensor_tensor(
                        out=t1[:], in0=t2[0:1, 0:1], in1=t2[0:1, 1:2], op=mx
                    )
                    return t1

                lrow = cpool.tile([1, E], F32)
                nc.vector.tensor_copy(lrow[:], lrow_pre[:])
                m1 = max_tree(cpool, lrow)
                eqm = cpool.tile([1, E], F32)
                nc.vector.tensor_tensor(
                    out=eqm[:], in0=lrow[:], in1=m1[:].to_broadcast([1, E]),
                    op=mybir.AluOpType.is_equal,
                )
                msk = cpool.tile([1, E], F32)
                nc.vector.tensor_mul(msk[:], eqm[:], cv_sb[:])
                mi = max_tree(cpool, msk)
                idxf = cpool.tile([1, 1], F32)
                # idx = (E-1) - mi
                nc.vector.tensor_scalar(
                    idxf[:], mi[:], -1.0, float(E - 1),
                    mybir.AluOpType.mult, mybir.AluOpType.add,
                )
                idxu = cpool.tile([1, 1], mybir.dt.uint32)
                nc.vector.tensor_copy(idxu[:], idxf[:])

                idx_regs = nc.alloc_registers(
                    "idx_regs", engines=[mybir.EngineType.Pool]
                )
                nc.regs_load(idx_regs, idxu[0:1, 0:1])
                idx = nc.snap(idx_regs, donate=True, min_val=0, max_val=E - 1)

                eb_sb = cpool.tile([128, NB], F32)
                nc.gpsimd.dma_start(
                    out=eb_sb[:], in_=ebs_d.ap()[bass.ds(idx * 128, 128), :]
                )

            # ---- gate GEMV: 4 x 2MB blocks ----
            gy = cpool.tile([128, NB], F32)
            for j in range(NB):
                wt = wepool.tile([128, H], F32, tag="we")
                nc.sync.dma_start(out=wt[:], in_=gw_v[:, j:j + 1, :])
                prod = ppool.tile([128, H + 8], F32, tag="prod")
                nc.vector.tensor_mul(
                    prod[:, 0:H], wt[:], x_bc[:, 0:H]
                )
                nc.scalar.activation(
                    prod[:, 0:H], prod[:, 0:H],
                    mybir.ActivationFunctionType.Copy,
                    accum_out=gy[:, j:j + 1],
                )

            # ---- expert GEMV: 2MB x3 + 1MB x2 at dynamic row offset ----
            # (fine granularity keeps DVE multiplies overlapped with the DMA
            #  stream; the 1MB tail halves shorten the post-DMA drain)
            ey = cpool.tile([128, NB], F32)
            r0 = idx * NB
            for j in range(3):
                wt = wepool.tile([128, H], F32, tag="we")
                nc.gpsimd.dma_start(
                    out=wt[:], in_=ew_v[:, bass.ds(r0 + j, 1), :]
                )
                prod = ppool.tile([128, H + 8], F32, tag="prod")
                nc.vector.tensor_mul(prod[:, 0:H], wt[:], x_bc[:, 0:H])
                nc.scalar.activation(
                    prod[:, 0:H], prod[:, 0:H],
                    mybir.ActivationFunctionType.Copy,
                    accum_out=ey[:, j:j + 1],
                )
            # block 3: two 1MB half-DMAs so the drain tail is short; the
            # second half reduces on DVE (idle right after its multiply)
            # so the final latency doesn't queue behind ACT's accumulates.
            eyh = cpool.tile([128, 2], F32)
            for c2 in range(2):
                wt3 = wepool.tile([128, H // 2], F32, tag="we")
                nc.gpsimd.dma_start(
                    out=wt3[:],
                    in_=ew_v[:, bass.ds(r0 + 3, 1), c2 * (H // 2):(c2 + 1) * (H // 2)],
                )
                prod3 = ppool.tile([128, H + 8], F32, tag="prod")
                nc.vector.tensor_mul(
                    prod3[:, 0:H // 2], wt3[:],
                    x_bc[:, c2 * (H // 2):(c2 + 1) * (H // 2)]
                )
                if c2 == 0:
                    nc.scalar.activation(
                        prod3[:, 0:H // 2], prod3[:, 0:H // 2],
                        mybir.ActivationFunctionType.Copy,
                        accum_out=eyh[:, c2:c2 + 1],
                    )
                else:
                    nc.vector.tensor_reduce(
                        out=eyh[:, c2:c2 + 1], in_=prod3[:, 0:H // 2],
                        axis=mybir.AxisListType.X, op=mybir.AluOpType.add,
                    )
            nc.vector.tensor_tensor(
                out=ey[:, 3:4], in0=eyh[:, 0:1], in1=eyh[:, 1:2],
                op=mybir.AluOpType.add,
            )

            # ---- tail: out = x + g * (tanh(ey + eb) - x) ----
            mix = cpool.tile([128, NB], F32)
            nc.vector.tensor_add(mix[:], ey[:], eb_sb[:])
            mix2 = cpool.tile([128, NB], F32)
            nc.scalar.activation(mix2[:], mix[:], mybir.ActivationFunctionType.Tanh)
            gsum = cpool.tile([128, NB], F32)
            nc.vector.tensor_add(gsum[:], gy[:], gbs_sb[:])
            g = cpool.tile([128, NB], F32)
            nc.scalar.activation(g[:], gsum[:], mybir.ActivationFunctionType.Sigmoid)
            d = cpool.tile([128, NB], F32)
            nc.vector.tensor_tensor(
                out=d[:], in0=mix2[:], in1=xs_sb[:], op=mybir.AluOpType.subtract
            )
            gd = cpool.tile([128, NB], F32)
            nc.vector.tensor_mul(gd[:], g[:], d[:])
            out_t = cpool.tile([128, NB], F32)
            nc.vector.tensor_add(out_t[:], xs_sb[:], gd[:])
            nc.sync.dma_start(out=yc_d.ap(), in_=out_t[:])

    if legalize:
        _legalize_single_wait(nc)
    return nc


def _as_f32(a):
    return np.ascontiguousarray(np.asarray(a, dtype=np.float32))


def kernel(x, expert_w, expert_b, router_w, router_b, gate_w, gate_b):
    global LAST_RESULTS
    x = _as_f32(x)
    expert_w = _as_f32(expert_w)
    expert_b = _as_f32(expert_b)
    router_w = _as_f32(router_w)
    router_b = _as_f32(router_b)
    gate_w = _as_f32(gate_w)
    gate_b = _as_f32(gate_b)

    if "nc" not in _CACHE:
        _CACHE["nc"] = _build_program()
    nc = _CACHE["nc"]

    xa = np.zeros((SEED, H + 8), np.float32)
    xa[:, 0:H] = x
    xa[:, H] = 1.0
    rwa = np.zeros((E, H + 8), np.float32)
    rwa[:, 0:H] = router_w
    rwa[:, H] = router_b
    in_maps = []
    for c in range(NCORES):
        sl = slice(c * S, (c + 1) * S)
        ew_c = np.ascontiguousarray(expert_w[:, sl, :]).reshape(E * S, H)
        ebs_c = np.ascontiguousarray(
            expert_b[:, sl].reshape(E, NB, 128).transpose(0, 2, 1)
        ).reshape(E * 128, NB)
        xs_c = np.ascontiguousarray(x[0, sl].reshape(NB, 128).T)
        gbs_c = np.ascontiguousarray(gate_b[sl].reshape(NB, 128).T)
        gw_c = np.ascontiguousarray(gate_w[sl, :])
        in_maps.append(
            {
                "x32_in": xa,
                "rw_in": rwa,
                "gw_in": gw_c,
                "ew_in": ew_c,
                "ebs_in": ebs_c,
                "xs_in": xs_c,
                "gbs_in": gbs_c,
            }
        )

    res = run_bass_kernel_spmd(nc, in_maps, core_ids=list(range(NCORES)))
    LAST_RESULTS = res

    y = np.empty((1, H), np.float32)
    for c in range(NCORES):
        yc = res.results[c]["yc_out"]  # [128, NB]; yc[p, j] = y[c*S + j*128 + p]
        y[0, c * S:(c + 1) * S] = yc.T.reshape(S)
    return y
